# revision 25
# baseline (speedup 1.0000x reference)
"""Trainium2 Bass kernel for batched per-item GRU cell.

Problem: nn_GRU_Cell — B=16, N=207 independent items, each with its own
C=64 -> 3H=192 weight matrices (Wx, Wh).  All ops are per-(b,n):

    xW          = x @ Wx                      [1, 192]
    r           = sigmoid(xW_r + h @ Wh_r + b_r)
    z           = sigmoid(xW_z + h @ Wh_z + b_z)
    hc          = tanh  (xW_c + (r*h) @ Wh_c + b_c)
    h_new       = (1 - z) * h + z * hc

Strategy (per core, items sharded 3312 -> 8 x 414):
  * Weights stream once from HBM: the r-gate in fp8e4m3 (its error is
    damped through r*h -> tanh -> z-blend; measured 5.8e-3 rel err vs
    the 2e-2 gate), z- and c-gates in bf16 (z feeds h_new directly and
    fails the gate in fp8).  16.96MB/core at the ~334-395GB/s/NC HBM
    rate = ~43-51us — the roofline.
  * Per item the weights are the PE *stationary* operand, K-stacked
    [x-block; h-block] (K=128), three matmuls per item:
      r:  S=[Wxr;Whr] fp8  [128,64] x [x;h]_fp8  -> psum_rz[0:64,  g]
      z:  S=[Wxz;Whz] bf16 [128,64] x [x;h]_bf16 -> psum_rz[64:128,g]
      c:  S=[Wxc;Whc] bf16 [128,64] x [x;r*h]    -> psum_c [0:64,  g]
    LDWEIGHTS cost is per-column, so the r/z split is free on the PE.
  * ALL activations/biases arrive HOST-TRANSPOSED in one DMA on the
    scalar queue; weights stream on the otherwise-empty sync queue as
    sub-DMAs of <=48 items (one continuous burst).
  * Global depth-2 software pipeline over subs: sub j's c-group is
    emitted after sub j+1's rz-group, hiding the psum->bias->sigmoid->
    r*h chain behind the next sub's matmuls.
  * h_new accumulates as [h, items] in one SBUF tile; ONE store at the
    end; the host transposes during unsharding (layout-only).
"""

import numpy as np

import concourse.bass as bass
import concourse.mybir as mybir
import concourse.tile as tile
from concourse import bacc
from concourse.bass_utils import run_bass_kernel_spmd

F32 = mybir.dt.float32
BF16 = mybir.dt.bfloat16
FP8 = mybir.dt.float8e4

B, N, C, H = 16, 207, 64, 64
J = 3 * H                  # 192
ITEMS = B * N              # 3312
NCORES = 8
PER = ITEMS // NCORES      # 414
CHUNKS = [96, 96, 96, 126]  # sum = 414
NCHUNK = len(CHUNKS)
GMAX = max(CHUNKS)
SUB = 48                   # sub-granule (weight DMA + compute pipeline)
SUB_LAST = 15              # finer pacing on the tail of the last chunk
AUXB = 3                   # aux blocks per chunk: xh_T | b_rz_T | b_c_T

AF = mybir.ActivationFunctionType


def _subs(k):
    G = CHUNKS[k]
    if k == NCHUNK - 1:
        # big subs first, then SUB_LAST-sized subs for the final 2*SUB_LAST
        cut = max(0, G - 2 * SUB_LAST)
        out = [(a, min(a + SUB, cut)) for a in range(0, cut, SUB)]
        out += [(a, min(a + SUB_LAST, G)) for a in range(cut, G, SUB_LAST)]
        return out
    return [(a, min(a + SUB, G)) for a in range(0, G, SUB)]


def build_nc(rdt=FP8):
    """Build the per-core Bass program.

    rdt: dtype of the r-gate weights + moving operand (FP8 or BF16).
    """
    # Bacc (not raw Bass): its compile() runs move_matmul_waits_to_ldweights
    # + generate_event_semaphores, which split multi-waits down to the 1-wait
    # ISA limit of PE instructions.
    nc = bacc.Bacc(None)
    rsz = mybir.dt.size(rdt)  # bytes per r-gate weight
    # aux, host-transposed and chunk-packed: for chunk k (offset q=3*s_k):
    #   cols [q      , q+G  ): xh_T  [128, G]  (x.T rows 0:64, h.T 64:128)
    #   cols [q+G    , q+2G ): brz_T [128, G]  (br.T rows 0:64, bz.T 64:128)
    #   cols [q+2G   , q+3G ): bc_T  [64, G]   (rows 0:64)
    aux_d = nc.declare_dram_parameter("aux", [128, AUXB * PER], F32,
                                      isOutput=False)
    # r-gate weights: per-chunk [c, item, j 0:64] blocks, flattened
    w8_d = nc.declare_dram_parameter("w8", [PER * 2 * C * 64], rdt,
                                     isOutput=False)
    # z+c weights: per-chunk [c, item, j 64:192] blocks, flattened
    w16_d = nc.declare_dram_parameter("w16", [PER * 2 * C * 128], BF16,
                                      isOutput=False)
    # h_new as [h, items] (host transposes once)
    out_d = nc.declare_dram_parameter("out", [H, PER], F32, isOutput=True)

    with tile.TileContext(nc) as tc:
        with (
            tc.tile_pool(name="const", bufs=1) as cpool,
            tc.tile_pool(name="w8", bufs=3) as w8pool,
            tc.tile_pool(name="w16", bufs=3) as w16pool,
            tc.tile_pool(name="act", bufs=2) as apool,
            tc.tile_pool(name="prz", bufs=2, space="PSUM") as prz_pool,
            tc.tile_pool(name="pc", bufs=2, space="PSUM") as pc_pool,
        ):
            # ---- aux rides the scalar HWDGE queue (its first op), off the
            # serial weight stream; SDMA interleaves both at packet level
            aux_all = cpool.tile([128, AUXB * PER], F32)
            nc.scalar.dma_start(out=aux_all[:], in_=aux_d[:, :])
            # all h_new columns accumulate here; ONE store at the end
            hn_all = cpool.tile([128, PER], F32)

            # Global depth-2 software pipeline over all subs: the c-group of
            # sub j is emitted AFTER the rz-group of sub j+1, so the
            # psum->bias->sigmoid->r*h chain of sub j hides behind sub
            # j+1's matmuls (PE executes in program order) and the ACT
            # queue never puts tanh(j) ahead of sigmoid(j+1).
            pending_c = None

            s = 0
            w8off = 0
            w16off = 0
            for k in range(NCHUNK):
                G = CHUNKS[k]
                q = AUXB * s

                # ---- this chunk's weights (sync queue, sub-DMAs) ---------
                # w8 [c(0:64)|c(64:128), item, j0:64]   = [Wxr ; Whr]
                # w16[c, item, j64:192]                 = [Wxz Wxc ; Whz Whc]
                w8 = w8pool.tile([128, GMAX, 64], rdt, tag="w8")
                w16 = w16pool.tile([128, GMAX, 128], BF16, tag="w16")
                w8src = w8_d[w8off:w8off + 128 * G * 64].rearrange(
                    "(c g j) -> c g j", c=128, g=G)
                w16src = w16_d[w16off:w16off + 128 * G * 128].rearrange(
                    "(c g j) -> c g j", c=128, g=G)
                for a, bb in _subs(k):
                    nc.sync.dma_start(
                        out=w8[:, a:bb, :], in_=w8src[:, a:bb, :])
                    nc.sync.dma_start(
                        out=w16[:, a:bb, :], in_=w16src[:, a:bb, :])

                xh = aux_all[:, q:q + G]
                b_rz = aux_all[:, q + G:q + 2 * G]
                b_c = aux_all[0:64, q + 2 * G:q + 3 * G]
                # casts run on the otherwise-idle gpsimd engine: on DVE they
                # would queue behind the PREVIOUS chunk's epilogue (FIFO) and
                # stall the next chunk's matmuls at the stream tail
                xh_m = apool.tile([128, G], BF16, tag="xh_m")
                nc.gpsimd.tensor_copy(xh_m[:], xh[:])
                xh_r = apool.tile([128, G], rdt, tag="xh_r")
                nc.gpsimd.tensor_copy(xh_r[:], xh[:])
                # c-pass moving columns: x half never changes, fill it now
                # (off the rz->sigmoid->r*h critical chain)
                rhs2 = apool.tile([128, G], BF16, tag="rhs2")
                nc.gpsimd.tensor_copy(rhs2[0:64, :], xh_m[0:64, :])

                psum_rz = prz_pool.tile([128, G], F32, tag="rz")
                psum_c = pc_pool.tile([128, G], F32, tag="c")
                t_rz = apool.tile([128, G], F32, tag="t_rz")
                rs = apool.tile([128, G], F32, tag="rs")
                zs = apool.tile([128, G], F32, tag="zs")
                t_c = apool.tile([128, G], F32, tag="t_c")
                hc = apool.tile([128, G], F32, tag="hc")
                e = apool.tile([128, G], F32, tag="e")
                f = apool.tile([128, G], F32, tag="f")

                for a, bb in _subs(k):
                    # ---- rz-group of this sub ----------------------------
                    for g in range(a, bb):
                        nc.tensor.matmul(
                            psum_rz[0:64, g:g + 1],
                            w8[:, g, :],
                            xh_r[:, g:g + 1],
                            start=True, stop=True,
                        )
                    for g in range(a, bb):
                        nc.tensor.matmul(
                            psum_rz[64:128, g:g + 1],
                            w16[:, g, 0:64],
                            xh_m[:, g:g + 1],
                            start=True, stop=True,
                        )
                    nc.vector.tensor_add(
                        t_rz[:, a:bb], psum_rz[:, a:bb], b_rz[:, a:bb])
                    # r evicted to rows 64:128 (ACT cross-offset) so r*h
                    # aligns with h there; z sigmoid runs off-chain
                    nc.scalar.activation(
                        rs[64:128, a:bb], t_rz[0:64, a:bb], AF.Sigmoid)
                    nc.scalar.activation(
                        zs[64:128, a:bb], t_rz[64:128, a:bb], AF.Sigmoid)
                    nc.vector.tensor_mul(
                        rhs2[64:128, a:bb], rs[64:128, a:bb], xh[64:128, a:bb])

                    # ---- previous sub's c-group --------------------------
                    if pending_c is not None:
                        pending_c()

                    def make_c(w16=w16, rhs2=rhs2, psum_c=psum_c, t_c=t_c,
                               hc=hc, e=e, f=f, b_c=b_c, zs=zs, xh=xh,
                               a=a, bb=bb, s=s):
                        def emit_c():
                            for g in range(a, bb):
                                nc.tensor.matmul(
                                    psum_c[0:64, g:g + 1],
                                    w16[:, g, 64:128],
                                    rhs2[:, g:g + 1],
                                    start=True, stop=True,
                                )
                            # ---- epilogue: hc, h_new = h + z*(hc - h) ----
                            nc.vector.tensor_add(
                                t_c[0:64, a:bb], psum_c[0:64, a:bb],
                                b_c[:, a:bb])
                            nc.scalar.activation(
                                hc[64:128, a:bb], t_c[0:64, a:bb], AF.Tanh)
                            nc.vector.tensor_sub(
                                e[64:128, a:bb], hc[64:128, a:bb],
                                xh[64:128, a:bb])
                            nc.vector.tensor_mul(
                                f[64:128, a:bb], zs[64:128, a:bb],
                                e[64:128, a:bb])
                            nc.vector.tensor_add(
                                hn_all[64:128, s + a:s + bb],
                                xh[64:128, a:bb], f[64:128, a:bb])
                        return emit_c

                    pending_c = make_c()

                s += G
                w8off += 128 * G * 64
                w16off += 128 * G * 128

            pending_c()

            # ---- ONE store of all h_new [h, items]; host transposes ------
            nc.scalar.dma_start(out=out_d[:, :], in_=hn_all[64:128, :])

    nc.compile()
    return nc


_CACHE = {}


def _get_nc(rdt):
    if rdt not in _CACHE:
        _CACHE[rdt] = build_nc(rdt)
    return _CACHE[rdt]


def _shards(x, state, Wx, Wh, b, rdt_np):
    import ml_dtypes
    x2 = np.asarray(x, np.float32).reshape(ITEMS, C)
    h2 = np.asarray(state, np.float32).reshape(ITEMS, H)
    b2 = np.asarray(b, np.float32).reshape(ITEMS, J)
    aux2 = np.ascontiguousarray(np.concatenate([x2, h2, b2], axis=1))
    wx2 = np.asarray(Wx).reshape(ITEMS, C, J)
    wh2 = np.asarray(Wh).reshape(ITEMS, H, J)
    w2 = np.concatenate([wx2, wh2], axis=1)          # [ITEMS, 2C, 192] f32
    w8 = w2[:, :, 0:64].astype(rdt_np).reshape(NCORES, PER, 2 * C, 64)
    w16 = w2[:, :, 64:192].astype(ml_dtypes.bfloat16).reshape(
        NCORES, PER, 2 * C, 128)
    aux3 = aux2.reshape(NCORES, PER, 2 * C + J)
    maps = []
    for i in range(NCORES):
        # aux host-transposed, chunk-packed: xh_T | brz_T | bc_T per chunk
        auxp = np.zeros((128, AUXB * PER), np.float32)
        s = 0
        for k, G in enumerate(CHUNKS):
            q = AUXB * s
            blockt = aux3[i, s:s + G].T          # [320, G]
            auxp[:, q:q + G] = blockt[0:128]          # x | h
            auxp[:, q + G:q + 2 * G] = blockt[128:256]    # br | bz
            auxp[0:64, q + 2 * G:q + 3 * G] = blockt[256:320]  # bc
            s += G
        # per chunk: [items, c, j] -> [c, item-in-chunk, j], flattened
        blocks8, blocks16 = [], []
        s = 0
        for G in CHUNKS:
            blocks8.append(w8[i, s:s + G].transpose(1, 0, 2).ravel())
            blocks16.append(w16[i, s:s + G].transpose(1, 0, 2).ravel())
            s += G
        maps.append({"aux": auxp,
                     "w8": np.concatenate(blocks8),
                     "w16": np.concatenate(blocks16)})
    return maps


def kernel(x, state, Wx, Wh, b, _trace=False, _rdt=FP8):
    import ml_dtypes
    rdt_np = {FP8: ml_dtypes.float8_e4m3,
              BF16: ml_dtypes.bfloat16}[_rdt]
    nc = _get_nc(_rdt)
    in_maps = _shards(x, state, Wx, Wh, b, rdt_np)
    res = run_bass_kernel_spmd(nc, in_maps, list(range(NCORES)), trace=_trace)
    # out: [H, PER] per core -> [ITEMS, H]
    out = np.concatenate(
        [res.results[i]["out"].T for i in range(NCORES)], axis=0)
    ret = np.ascontiguousarray(out).reshape(B, N, 1, H)
    if _trace:
        return ret, res
    return ret


# revision 37
# speedup vs baseline: 1.3565x; 1.3565x over previous
"""Trainium2 Bass kernel for batched per-item GRU cell.

Problem: nn_GRU_Cell — B=16, N=207 independent items, each with its own
C=64 -> 3H=192 weight matrices (Wx, Wh).  All ops are per-(b,n):

    xW          = x @ Wx                      [1, 192]
    r           = sigmoid(xW_r + h @ Wh_r + b_r)
    z           = sigmoid(xW_z + h @ Wh_z + b_z)
    hc          = tanh  (xW_c + (r*h) @ Wh_c + b_c)
    h_new       = (1 - z) * h + z * hc

Strategy (per core, items sharded 3312 -> 8 x 414):
  * Weights stream once from HBM: the r-gate in fp8e4m3 (its error is
    damped through r*h -> tanh -> z-blend; measured 5.8e-3 rel err vs
    the 2e-2 gate), z- and c-gates in bf16 (z feeds h_new directly and
    fails the gate in fp8).  16.96MB/core at the ~334-395GB/s/NC HBM
    rate = ~43-51us — the roofline.
  * Per item the weights are the PE *stationary* operand, K-stacked
    [x-block; h-block] (K=128).  To keep the PE at 2 matmul calls per
    item (per-instruction overhead dominates at 1-column moving size),
    the r- and c-matmuls are PAIRED across adjacent items:
      r-pair: S=[r-wts(g)|r-wts(g+1)] fp8 [128,128] x 2 cols [x;h]_fp8
              -> psum_rz[:, g:g+2] (valid r(g)@rows 0:64 of col g,
                 r(g+1)@rows 64:128 of col g+1; garbage elsewhere)
      z:      S=[Wxz;Whz] bf16 [128,64] x [x;h]_bf16, overwriting the
              garbage half: col even -> rows 64:128, col odd -> 0:64
      c-pair: like r-pair with [x;r*h] moving -> psum_c, valid halves
              alternating; biases arrive pre-alternated from the host
              so the +bias stays ONE op and the sigmoids/tanh use
              stride-2 APs to normalize everything onto rows 64:128.
    LDWEIGHTS cost is per-column (192/item either way), so pairing
    only removes instruction overhead.
  * ALL activations/biases arrive HOST-TRANSPOSED in one DMA on the
    scalar queue; weights stream on the otherwise-empty sync queue as
    sub-DMAs of <=48 items (one continuous burst).
  * Global depth-2 software pipeline over subs: sub j's c-group is
    emitted after sub j+1's rz-group, hiding the psum->bias->sigmoid->
    r*h chain behind the next sub's matmuls.
  * h_new accumulates as [h, items] in one SBUF tile; ONE store at the
    end; the host transposes during unsharding (layout-only).
"""

import numpy as np

import concourse.bass as bass
import concourse.mybir as mybir
import concourse.tile as tile
from concourse import bacc
from concourse.bass_utils import run_bass_kernel_spmd

F32 = mybir.dt.float32
BF16 = mybir.dt.bfloat16
FP8 = mybir.dt.float8e4

B, N, C, H = 16, 207, 64, 64
J = 3 * H                  # 192
ITEMS = B * N              # 3312
NCORES = 8
PER = ITEMS // NCORES      # 414
CHUNKS = [96, 96, 96, 126]  # sum = 414
NCHUNK = len(CHUNKS)
GMAX = max(CHUNKS)
SUB = 48                   # sub-granule (weight DMA + compute pipeline)
SUB_LAST = 16              # finer pacing on the tail (even: items pair up)
AUXB = 3                   # aux blocks per chunk: xh_T | b_rz_T | b_c_T

AF = mybir.ActivationFunctionType


def _subs(k):
    G = CHUNKS[k]
    if k == NCHUNK - 1:
        # big subs first, then SUB_LAST-sized subs for the final 2*SUB_LAST
        cut = max(0, G - 2 * SUB_LAST)
        out = [(a, min(a + SUB, cut)) for a in range(0, cut, SUB)]
        out += [(a, min(a + SUB_LAST, G)) for a in range(cut, G, SUB_LAST)]
        return out
    return [(a, min(a + SUB, G)) for a in range(0, G, SUB)]


def build_nc(rdt=FP8):
    """Build the per-core Bass program.

    rdt: dtype of the r-gate weights + moving operand (FP8 or BF16).
    """
    # Bacc (not raw Bass): its compile() runs move_matmul_waits_to_ldweights
    # + generate_event_semaphores, which split multi-waits down to the 1-wait
    # ISA limit of PE instructions.
    nc = bacc.Bacc(None)
    rsz = mybir.dt.size(rdt)  # bytes per r-gate weight
    # aux, host-transposed and chunk-packed: for chunk k (offset q=3*s_k):
    #   cols [q      , q+G  ): xh_T  [128, G]  (x.T rows 0:64, h.T 64:128)
    #   cols [q+G    , q+2G ): brz_T [128, G]  (parity-alternated r/z bias)
    #   cols [q+2G   , q+3G ): bc_T  [128, G]  (bc duplicated on both halves)
    aux_d = nc.declare_dram_parameter("aux", [128, AUXB * PER], F32,
                                      isOutput=False)
    # r-gate weights: per-chunk [c, item, j 0:64] blocks, flattened
    w8_d = nc.declare_dram_parameter("w8", [PER * 2 * C * 64], rdt,
                                     isOutput=False)
    # z+c weights, pair-major so the paired c-matmul's stationary is one
    # contiguous 128-col run: per chunk [c, pair, z(2p)|z(2p+1)|c(2p)|c(2p+1)]
    w16_d = nc.declare_dram_parameter("w16", [PER * 2 * C * 128], BF16,
                                      isOutput=False)
    # h_new as [h, items] (host transposes once)
    out_d = nc.declare_dram_parameter("out", [H, PER], F32, isOutput=True)

    with tile.TileContext(nc) as tc:
        with (
            tc.tile_pool(name="const", bufs=1) as cpool,
            tc.tile_pool(name="w8", bufs=3) as w8pool,
            tc.tile_pool(name="w16", bufs=3) as w16pool,
            tc.tile_pool(name="act", bufs=2) as apool,
            tc.tile_pool(name="prz", bufs=2, space="PSUM") as prz_pool,
            tc.tile_pool(name="pc", bufs=2, space="PSUM") as pc_pool,
        ):
            # ---- aux rides the scalar HWDGE queue (its first op), off the
            # serial weight stream; SDMA interleaves both at packet level
            aux_all = cpool.tile([128, AUXB * PER], F32)
            nc.scalar.dma_start(out=aux_all[:], in_=aux_d[:, :])
            # all h_new columns accumulate here; ONE store at the end
            hn_all = cpool.tile([128, PER], F32)

            # Global depth-2 software pipeline over all subs: the c-group of
            # sub j is emitted AFTER the rz-group of sub j+1, so the
            # psum->bias->sigmoid->r*h chain of sub j hides behind sub
            # j+1's matmuls (PE executes in program order) and the ACT
            # queue never puts tanh(j) ahead of sigmoid(j+1).
            pending_c = None

            s = 0
            w8off = 0
            w16off = 0
            for k in range(NCHUNK):
                G = CHUNKS[k]
                q = AUXB * s

                # ---- this chunk's weights (sync queue, sub-DMAs) ---------
                # w8 [c(0:64)|c(64:128), item, j0:64]   = [Wxr ; Whr]
                # w16[c, item, j64:192]                 = [Wxz Wxc ; Whz Whc]
                w8 = w8pool.tile([128, GMAX, 64], rdt, tag="w8")
                w16 = w16pool.tile([128, GMAX // 2, 256], BF16, tag="w16")
                w8src = w8_d[w8off:w8off + 128 * G * 64].rearrange(
                    "(c g j) -> c g j", c=128, g=G)
                w16src = w16_d[w16off:w16off + 128 * G * 128].rearrange(
                    "(c p j) -> c p j", c=128, p=G // 2)
                for a, bb in _subs(k):
                    nc.sync.dma_start(
                        out=w8[:, a:bb, :], in_=w8src[:, a:bb, :])
                    nc.sync.dma_start(
                        out=w16[:, a // 2:bb // 2, :],
                        in_=w16src[:, a // 2:bb // 2, :])

                xh = aux_all[:, q:q + G]
                b_rz = aux_all[:, q + G:q + 2 * G]
                b_c = aux_all[:, q + 2 * G:q + 3 * G]
                # casts run on the otherwise-idle gpsimd engine: on DVE they
                # would queue behind the PREVIOUS chunk's epilogue (FIFO) and
                # stall the next chunk's matmuls at the stream tail
                xh_m = apool.tile([128, G], BF16, tag="xh_m")
                nc.gpsimd.tensor_copy(xh_m[:], xh[:])
                xh_r = apool.tile([128, G], rdt, tag="xh_r")
                nc.gpsimd.tensor_copy(xh_r[:], xh[:])
                # c-pass moving columns: x half never changes, fill it now
                # (off the rz->sigmoid->r*h critical chain)
                rhs2 = apool.tile([128, G], BF16, tag="rhs2")
                nc.gpsimd.tensor_copy(rhs2[0:64, :], xh_m[0:64, :])

                psum_rz = prz_pool.tile([128, G], F32, tag="rz")
                psum_c = pc_pool.tile([128, G], F32, tag="c")
                t_rz = apool.tile([128, G], F32, tag="t_rz")
                rs = apool.tile([128, G], F32, tag="rs")
                zs = apool.tile([128, G], F32, tag="zs")
                t_c = apool.tile([128, G], F32, tag="t_c")
                hc = apool.tile([128, G], F32, tag="hc")
                e = apool.tile([128, G], F32, tag="e")
                f = apool.tile([128, G], F32, tag="f")

                for a, bb in _subs(k):
                    # ---- rz-group of this sub ----------------------------
                    # r-pairs write FULL [128, 2] columns (garbage in the
                    # off-half), then z-singles overwrite the garbage half
                    for g in range(a, bb, 2):
                        nc.tensor.matmul(
                            psum_rz[:, g:g + 2],
                            w8[:, g:g + 2, :],
                            xh_r[:, g:g + 2],
                            start=True, stop=True,
                        )
                    for g in range(a, bb):
                        half = slice(64, 128) if g % 2 == 0 else slice(0, 64)
                        zoff = (g % 2) * 64
                        nc.tensor.matmul(
                            psum_rz[half, g:g + 1],
                            w16[:, g // 2, zoff:zoff + 64],
                            xh_m[:, g:g + 1],
                            start=True, stop=True,
                        )
                    # bias arrives host-pre-alternated to match the column
                    # parity layout, so this stays ONE op
                    nc.vector.tensor_add(
                        t_rz[:, a:bb], psum_rz[:, a:bb], b_rz[:, a:bb])
                    # normalize r and z onto rows 64:128 (stride-2 per
                    # parity; ACT handles cross-partition-offset moves)
                    nc.scalar.activation(
                        rs[64:128, a:bb:2], t_rz[0:64, a:bb:2], AF.Sigmoid)
                    nc.scalar.activation(
                        rs[64:128, a + 1:bb:2], t_rz[64:128, a + 1:bb:2],
                        AF.Sigmoid)
                    nc.scalar.activation(
                        zs[64:128, a:bb:2], t_rz[64:128, a:bb:2], AF.Sigmoid)
                    nc.scalar.activation(
                        zs[64:128, a + 1:bb:2], t_rz[0:64, a + 1:bb:2],
                        AF.Sigmoid)
                    nc.vector.tensor_mul(
                        rhs2[64:128, a:bb], rs[64:128, a:bb], xh[64:128, a:bb])

                    # ---- previous sub's c-group --------------------------
                    if pending_c is not None:
                        pending_c()

                    def make_c(w16=w16, rhs2=rhs2, psum_c=psum_c, t_c=t_c,
                               hc=hc, e=e, f=f, b_c=b_c, zs=zs, xh=xh,
                               a=a, bb=bb, s=s):
                        def emit_c():
                            for g in range(a, bb, 2):
                                nc.tensor.matmul(
                                    psum_c[:, g:g + 2],
                                    w16[:, g // 2, 128:256],
                                    rhs2[:, g:g + 2],
                                    start=True, stop=True,
                                )
                            # ---- epilogue: hc, h_new = h + z*(hc - h) ----
                            # valid c halves alternate; b_c is duplicated on
                            # both halves host-side
                            nc.vector.tensor_add(
                                t_c[0:64, a:bb:2], psum_c[0:64, a:bb:2],
                                b_c[0:64, a:bb:2])
                            nc.vector.tensor_add(
                                t_c[64:128, a + 1:bb:2],
                                psum_c[64:128, a + 1:bb:2],
                                b_c[64:128, a + 1:bb:2])
                            nc.scalar.activation(
                                hc[64:128, a:bb:2], t_c[0:64, a:bb:2],
                                AF.Tanh)
                            nc.scalar.activation(
                                hc[64:128, a + 1:bb:2],
                                t_c[64:128, a + 1:bb:2], AF.Tanh)
                            nc.vector.tensor_sub(
                                e[64:128, a:bb], hc[64:128, a:bb],
                                xh[64:128, a:bb])
                            nc.vector.tensor_mul(
                                f[64:128, a:bb], zs[64:128, a:bb],
                                e[64:128, a:bb])
                            nc.vector.tensor_add(
                                hn_all[64:128, s + a:s + bb],
                                xh[64:128, a:bb], f[64:128, a:bb])
                        return emit_c

                    pending_c = make_c()

                s += G
                w8off += 128 * G * 64
                w16off += 128 * G * 128

            pending_c()

            # ---- ONE store of all h_new [h, items]; host transposes ------
            nc.scalar.dma_start(out=out_d[:, :], in_=hn_all[64:128, :])

    nc.compile()
    return nc


_CACHE = {}


def _get_nc(rdt):
    if rdt not in _CACHE:
        _CACHE[rdt] = build_nc(rdt)
    return _CACHE[rdt]


def _shards(x, state, Wx, Wh, b, rdt_np):
    import ml_dtypes
    x2 = np.asarray(x, np.float32).reshape(ITEMS, C)
    h2 = np.asarray(state, np.float32).reshape(ITEMS, H)
    b2 = np.asarray(b, np.float32).reshape(ITEMS, J)
    aux2 = np.ascontiguousarray(np.concatenate([x2, h2, b2], axis=1))
    wx2 = np.asarray(Wx).reshape(ITEMS, C, J)
    wh2 = np.asarray(Wh).reshape(ITEMS, H, J)
    w2 = np.concatenate([wx2, wh2], axis=1)          # [ITEMS, 2C, 192] f32
    w8 = w2[:, :, 0:64].astype(rdt_np).reshape(NCORES, PER, 2 * C, 64)
    wz = w2[:, :, 64:128].astype(ml_dtypes.bfloat16).reshape(
        NCORES, PER, 2 * C, 64)
    wc = w2[:, :, 128:192].astype(ml_dtypes.bfloat16).reshape(
        NCORES, PER, 2 * C, 64)
    aux3 = aux2.reshape(NCORES, PER, 2 * C + J)
    maps = []
    for i in range(NCORES):
        # aux host-transposed, chunk-packed: xh_T | brz_T | bc_T per chunk
        auxp = np.zeros((128, AUXB * PER), np.float32)
        s = 0
        for k, G in enumerate(CHUNKS):
            q = AUXB * s
            blockt = aux3[i, s:s + G].T          # [320, G]
            auxp[:, q:q + G] = blockt[0:128]          # x | h
            # r/z bias alternated to match the paired-matmul psum layout:
            # even cols [br; bz], odd cols [bz; br]
            brz = blockt[128:256].copy()              # [128, G] = [br; bz]
            brz[:, 1::2] = np.concatenate(
                [blockt[192:256, 1::2], blockt[128:192, 1::2]], axis=0)
            auxp[:, q + G:q + 2 * G] = brz
            # bc on both halves (even cols read 0:64, odd cols 64:128)
            auxp[0:64, q + 2 * G:q + 3 * G] = blockt[256:320]
            auxp[64:128, q + 2 * G:q + 3 * G] = blockt[256:320]
            s += G
        # w8 per chunk: [items, c, j] -> [c, item, j]; w16 pair-major:
        # [c, pair, z(2p)|z(2p+1)|c(2p)|c(2p+1)]
        blocks8, blocks16 = [], []
        s = 0
        for G in CHUNKS:
            blocks8.append(w8[i, s:s + G].transpose(1, 0, 2).ravel())
            z = wz[i, s:s + G].reshape(G // 2, 2, 2 * C, 64)
            c = wc[i, s:s + G].reshape(G // 2, 2, 2 * C, 64)
            pair = np.concatenate(
                [z[:, 0], z[:, 1], c[:, 0], c[:, 1]], axis=-1)  # [P,2C,256]
            blocks16.append(pair.transpose(1, 0, 2).ravel())
            s += G
        maps.append({"aux": auxp,
                     "w8": np.concatenate(blocks8),
                     "w16": np.concatenate(blocks16)})
    return maps


def kernel(x, state, Wx, Wh, b, _trace=False, _rdt=FP8):
    import ml_dtypes
    rdt_np = {FP8: ml_dtypes.float8_e4m3,
              BF16: ml_dtypes.bfloat16}[_rdt]
    nc = _get_nc(_rdt)
    in_maps = _shards(x, state, Wx, Wh, b, rdt_np)
    res = run_bass_kernel_spmd(nc, in_maps, list(range(NCORES)), trace=_trace)
    # out: [H, PER] per core -> [ITEMS, H]
    out = np.concatenate(
        [res.results[i]["out"].T for i in range(NCORES)], axis=0)
    ret = np.ascontiguousarray(out).reshape(B, N, 1, H)
    if _trace:
        return ret, res
    return ret
